# revision 29
# baseline (speedup 1.0000x reference)
"""Deformable conv2d (torchvision semantics: stride=1, pad=0, dil=1,
offset_groups=1, no mask/bias) on 8 TRN2 NeuronCores via Bass/Tile.

Hardcoded shapes: input [16,64,66,66] f32, offset [16,18,64,64] f32,
weight [64,64,3,3] f32 -> out [16,64,64,64] f32.

Sharding: data-parallel over batch; core i handles images (2i, 2i+1).

Per-core scheme (per image):
  - HBM gather table per image: row m = tx*68+ty (tx-major), 128 bf16
    entries [c*2+x] = I[c, ty-1, tx-1+x] (zero-padded halo).  A SWDGE
    dma_gather (transpose mode) per (chunk t, image) fetches, for each
    of 4608 = 9 taps x 512 pixel indices, the 512B row-PAIR (rows m,
    m+1 = both y-corners), transposed so SBUF partitions carry (c,x)
    and the free dim carries (y-corner j, index position).
  - idx m = clamp(x0+1,0,67)*68 + clamp(y0+1,0,67); the zero halo makes
    every out-of-bounds corner read exactly 0, except the x1 slot when
    x0 < -1 and the second row when y0 < -1, which are killed in the
    bilinear weights (X1 *= [x0>=-1], Y1 *= [y0>=-1]).
  - bilinear corner weight products P[i,j] are computed compactly on
    DVE in a (t, k)-partition layout, staged contiguously through DRAM
    and read back as P_B [128=(ke,i,j,im,kh), 4096=(t,q,r)] so tiny
    one-hot contraction-64 matmuls broadcast them to the (c,x)
    partition layout (f32 PSUM), drained to bf16 by ScalarE.
  - DVE: S = G * W (bf16); TensorE: per tap 2 matmuls (j=0/1)
    contracting (c,x)=128, accumulating 9 taps in PSUM.
"""

import sys

sys.path.insert(0, "/opt/trn_rl_repo")

import ml_dtypes
import numpy as np

import concourse.bacc as bacc
import concourse.bass as bass
import concourse.mybir as mybir
import concourse.tile as tile

F32 = mybir.dt.float32
F32R = mybir.dt.float32r
BF16 = mybir.dt.bfloat16
I16 = mybir.dt.int16
I32 = mybir.dt.int32

N, CIN, COUT = 16, 64, 64
HIN, WIN = 66, 66
KH, KW = 3, 3
HO, WO = 64, 64
K = KH * KW
NPX = HO * WO  # 4096
NCORES = 8

TE = 68
NROW_PAD = 4640
NT = 8  # pixel chunks
CHUNK = 512
NIDX = K * CHUNK  # 4608 indices per (chunk, image) gather
CCOL = 2 * K * 32  # 576 idx-side compact cols: (im, k, qq)
N_COPY_KH = 3  # kh-blocks per (t,im) routed via Scalar PSUM->bf16 copy


def _alu(name):
    return getattr(mybir.AluOpType, name)


def build_bass(num_devices=NCORES):
    nc = bacc.Bacc("TRN2", target_bir_lowering=False, debug=False,
                   num_devices=num_devices)

    din = {}
    for nm, shp, dt in [
        ("taba", [NROW_PAD, 128], BF16), ("tabb", [NROW_PAD, 128], BF16),
        ("dyx", [128, 2 * CCOL], F32), ("byx", [128, 2 * CCOL], F32),
        ("dyx2", [128, 2048], F32), ("byx2", [128, 2048], F32),
        ("wmat2", [128, K * 64], BF16), ("sel", [128, 20 * 128], F32),
    ]:
        din[nm] = nc.dram_tensor(nm, shp, dt, kind="ExternalInput")
    out_d = nc.dram_tensor("out128", [128, NPX], F32, kind="ExternalOutput")
    istage = nc.dram_tensor("istage", [128 * CCOL], I16)
    wstage = nc.dram_tensor("wstage", [128 * 4096], BF16)

    # overlapping-window AP over each table: idx m -> 256 elems (rows m, m+1)
    tab_aps = [
        bass.AP(din[nm], 0, [[128, NROW_PAD - 1], [1, 256]])
        for nm in ("taba", "tabb")
    ]

    with tile.TileContext(nc) as tc:
        with tc.tile_pool(name="cst", bufs=1) as cpool:
            idxw = cpool.tile([128, 4608], I16, name="idxw")
            pb = cpool.tile([128, 4096], BF16, name="pb")
            wmat2 = cpool.tile([128, K * 64], BF16, name="wmat2_t")
            selt = cpool.tile([128, 20 * 128], BF16, name="sel_t")

            # ---------- phase 1: index + weight pipelines ----------
            with tc.tile_pool(name="pipe", bufs=1) as tp:
                sel_f = tp.tile([128, 20 * 128], F32, tag="sf", name="sel_f")
                nc.sync.dma_start(sel_f[:], din["sel"].ap())
                nc.vector.tensor_copy(selt[:], sel_f[:])
                nc.sync.dma_start(wmat2[:], din["wmat2"].ap())
                nc.gpsimd.memset(pb[:], 0.0)

                def floor_(src, dst, nf, pool):
                    ti = pool.tile([128, nf], I32, tag="ti", name=f"ti{nf}")
                    nc.vector.tensor_copy(ti[:], src[:])
                    tf = pool.tile([128, nf], F32, tag="tf", name=f"tf{nf}")
                    nc.vector.tensor_copy(tf[:], ti[:])
                    co = pool.tile([128, nf], F32, tag="co", name=f"co{nf}")
                    nc.vector.tensor_tensor(co[:], tf[:], src[:],
                                            _alu("is_gt"))
                    nc.vector.tensor_sub(dst[:], tf[:], co[:])

                # --- idx chain, layout A: v=(t,r), col=(im,k,q) ---
                W2 = 2 * CCOL
                dyx = tp.tile([128, W2], F32, tag="dyx", name="dyx_t")
                nc.sync.dma_start(dyx[:], din["dyx"].ap())
                byx = tp.tile([128, W2], F32, tag="byx", name="byx_t")
                nc.sync.dma_start(byx[:], din["byx"].ap())
                pos = tp.tile([128, W2], F32, tag="pos", name="pos_t")
                nc.vector.tensor_add(pos[:], byx[:], dyx[:])
                fl = tp.tile([128, W2], F32, tag="fl", name="fl_t")
                floor_(pos, fl, W2, tp)
                tcl = tp.tile([128, W2], F32, tag="tcl", name="tcl_t")
                nc.vector.tensor_scalar(tcl[:], fl[:], 1.0, 67.0,
                                        _alu("add"), _alu("min"))
                nc.vector.tensor_scalar(tcl[:], tcl[:], 0.0, None, _alu("max"))
                mf = tp.tile([128, CCOL], F32, tag="mf", name="mf_t")
                nc.vector.tensor_scalar(mf[:], tcl[:, CCOL:W2], 68.0, None,
                                        _alu("mult"))
                nc.vector.tensor_add(mf[:], mf[:], tcl[:, 0:CCOL])
                icomp = tp.tile([128, CCOL], I16, tag="ic", name="icomp_t")
                nc.vector.tensor_copy(icomp[:], mf[:])

                nc.sync.dma_start(
                    istage.ap().rearrange("(v c) -> v c", c=CCOL), icomp[:])
                isrc = bass.AP(istage, 0, [[576, 16], [9216, 8], [1, 576]])
                for grp in range(8):
                    nc.sync.dma_start(idxw[grp * 16:(grp + 1) * 16, :], isrc)

                # --- weight chain, layout A2: v=(t,kp16), col=(im,q,r) ---
                W4 = 2048
                dyx2 = tp.tile([128, W4], F32, tag="dyx2", name="dyx2_t")
                nc.sync.dma_start(dyx2[:], din["dyx2"].ap())
                byx2 = tp.tile([128, W4], F32, tag="byx2", name="byx2_t")
                nc.sync.dma_start(byx2[:], din["byx2"].ap())
                pos2 = tp.tile([128, W4], F32, tag="pos2", name="pos2_t")
                nc.vector.tensor_add(pos2[:], byx2[:], dyx2[:])
                fl2 = tp.tile([128, W4], F32, tag="fl2", name="fl2_t")
                floor_(pos2, fl2, W4, tp)
                fr2 = tp.tile([128, W4], F32, tag="fr2", name="fr2_t")
                nc.vector.tensor_sub(fr2[:], pos2[:], fl2[:])
                msk = tp.tile([128, W4], F32, tag="msk", name="msk_t")
                nc.vector.tensor_scalar(msk[:], fl2[:], -1.0, None,
                                        _alu("is_ge"))
                f1 = tp.tile([128, W4], F32, tag="f1", name="f1_t")
                nc.vector.tensor_mul(f1[:], fr2[:], msk[:])
                f0 = tp.tile([128, W4], F32, tag="f0", name="f0_t")
                nc.vector.tensor_scalar(f0[:], fr2[:], -1.0, 1.0,
                                        _alu("mult"), _alu("add"))
                # products: prod[:, (i*2+j)*1024:+1024] = X_i * Y_j
                prod = tp.tile([128, 4096], BF16, tag="prod", name="prod_t")
                xs = [f0[:, 1024:2048], f1[:, 1024:2048]]
                ys = [f0[:, 0:1024], f1[:, 0:1024]]
                for i in range(2):
                    for j in range(2):
                        ij = i * 2 + j
                        nc.vector.tensor_tensor(
                            prod[:, ij * 1024:(ij + 1) * 1024],
                            xs[i], ys[j], _alu("mult"))

                nc.sync.dma_start(
                    wstage.ap().rearrange("(v c) -> v c", c=4096), prod[:])
                # reads: pb[ke*64+i*32+j*16+im*8+kh, (t, q, r)]
                # wstage flat: t:65536, kp:4096, i:2048, j:1024, im:512,
                #              q:16, r:1 ; k = 2*kh + ke -> kh stride 8192
                for ke in range(2):
                    nkh = 5 if ke == 0 else 4
                    for i in range(2):
                        for j in range(2):
                            for im in range(2):
                                base = ke * 64 + i * 32 + j * 16 + im * 8
                                off0 = (ke * 4096 + i * 2048 + j * 1024
                                        + im * 512)
                                src = bass.AP(
                                    wstage, off0,
                                    [[8192, nkh], [65536, 8], [1, 512]])
                                nc.sync.dma_start(pb[base:base + nkh, :], src)

            # ---------- phase 2: main loop ----------
            with tc.tile_pool(name="gat", bufs=20) as gpool, \
                 tc.tile_pool(name="smul", bufs=22) as spool, \
                 tc.tile_pool(name="wbp", bufs=4) as wbpool, \
                 tc.tile_pool(name="outs", bufs=2) as opool, \
                 tc.tile_pool(name="wps", bufs=3, space="PSUM") as wps, \
                 tc.tile_pool(name="ops", bufs=2, space="PSUM") as ops_:
                for t in range(NT):
                    for im in range(2):
                        gt = {}
                        for k in range(K):
                            g = gpool.tile([128, 2, CHUNK], BF16, tag="g",
                                           name=f"g_{t}_{im}_{k}")
                            nc.gpsimd.dma_gather(
                                g[:], tab_aps[im],
                                idxw[:, (t * 2 + im) * 288 + k * 32:
                                     (t * 2 + im) * 288 + (k + 1) * 32],
                                CHUNK, CHUNK, elem_size=256, elem_step=128,
                                transpose=True, single_packet=False)
                            gt[k] = g
                        out_ps = ops_.tile([64, CHUNK], F32, tag="ops",
                                           name=f"ops_{t}_{im}")
                        S = {}
                        for kh in range(5):
                            nke = 2 if kh < 4 else 1
                            for j in range(2):
                                v20 = im * 10 + j * 5 + kh
                                wp = wps.tile([128, 2, CHUNK], F32, tag="wp",
                                              name=f"wp_{t}_{im}_{kh}_{j}")
                                for ke in range(nke):
                                    nc.tensor.matmul(
                                        wp[:, ke, :],
                                        selt[ke * 64:(ke + 1) * 64,
                                             v20 * 128:(v20 + 1) * 128],
                                        pb[ke * 64:(ke + 1) * 64,
                                           t * CHUNK:(t + 1) * CHUNK],
                                        start=True, stop=True)
                                if kh < N_COPY_KH:
                                    wpb = wbpool.tile(
                                        [128, 2, CHUNK], BF16, tag="wpb",
                                        name=f"wpb_{t}_{im}_{kh}_{j}")
                                    wpbf = wpb[:].rearrange(
                                        "p a b -> p (a b)")
                                    wpf = wp[:].rearrange("p a b -> p (a b)")
                                    nc.scalar.copy(
                                        wpbf[:, 0:nke * CHUNK],
                                        wpf[:, 0:nke * CHUNK])
                                    srcw = wpb
                                else:
                                    srcw = wp
                                for ke in range(nke):
                                    k = 2 * kh + ke
                                    s = spool.tile(
                                        [128, CHUNK], BF16, tag="s",
                                        name=f"s_{t}_{im}_{k}_{j}")
                                    nc.vector.tensor_tensor(
                                        s[:], gt[k][:, j, :],
                                        srcw[:, ke, :], _alu("mult"))
                                    S[(k, j)] = s
                        for kh in range(5):
                            nke = 2 if kh < 4 else 1
                            for ke in range(nke):
                                k = 2 * kh + ke
                                for j in range(2):
                                    nc.tensor.matmul(
                                        out_ps[:, :],
                                        wmat2[:, k * 64:(k + 1) * 64],
                                        S[(k, j)][:],
                                        start=(k == 0 and j == 0),
                                        stop=(k == 8 and j == 1))
                        ot = opool.tile([64, CHUNK], F32, tag="ot",
                                        name=f"ot_{t}_{im}")
                        nc.scalar.copy(ot[:], out_ps[:, :])
                        nc.sync.dma_start(
                            out_d.ap()[im * 64:(im + 1) * 64,
                                       t * CHUNK:(t + 1) * CHUNK], ot[:])

    nc.compile()
    return nc


# ---------------- host side ----------------

def _build_tables(inp):
    """inp [N, 64, 66, 66] f32 -> per-image [NROW_PAD, 128] bf16 tables."""
    n = inp.shape[0]
    ipad = np.zeros((n, 64, TE, TE + 1), np.float32)
    ipad[:, :, 1:1 + HIN, 1:1 + WIN] = inp
    # tab[n, tx, ty, c, i] = ipad[n, c, ty, tx+i]
    tab = np.empty((n, TE, TE, 64, 2), np.float32)
    tr = ipad.transpose(0, 3, 2, 1)  # [n, b(=tx+i), ty, c]
    tab[..., 0] = tr[:, 0:TE].transpose(0, 1, 2, 3)
    tab[..., 1] = tr[:, 1:TE + 1]
    full = np.zeros((n, NROW_PAD, 128), ml_dtypes.bfloat16)
    full[:, :TE * TE] = tab.reshape(n, TE * TE, 128).astype(
        ml_dtypes.bfloat16)
    return full


def _host_arrays(input, offset, weight):
    inp = np.ascontiguousarray(input, dtype=np.float32)
    off = np.ascontiguousarray(offset, dtype=np.float32)
    w = np.ascontiguousarray(weight, dtype=np.float32)

    tabs = _build_tables(inp)

    # wmat2[c*2+i, k*64+o] = w[o, c, kh, kw]
    wck_o = w.reshape(COUT, CIN, K).transpose(1, 2, 0)  # [c, k, o]
    wmat2 = np.broadcast_to(wck_o[:, None], (64, 2, K, 64))
    wmat2 = np.ascontiguousarray(wmat2.reshape(128, K * 64),
                                 dtype=ml_dtypes.bfloat16)

    # sel[z, v20*128 + u] = 1 iff z == (u%2)*32 + j*16 + im*8 + kh
    sel = np.zeros((64, 20, 128), np.float32)
    for im in range(2):
        for j in range(2):
            for kh in range(5):
                v20 = im * 10 + j * 5 + kh
                z0 = j * 16 + im * 8 + kh
                sel[z0, v20, 0::2] = 1.0
                sel[z0 + 32, v20, 1::2] = 1.0
    sel = sel.reshape(64, 20 * 128)
    sel = np.concatenate([sel, sel], axis=0)  # replicate for base 64 slices

    kh_of = (np.arange(K) // KW).astype(np.float32)
    kw_of = (np.arange(K) % KW).astype(np.float32)

    # layout A (idx): v = t*16 + r, col = im*288 + k*32 + q,
    # pixel p = t*512 + q*16 + r
    t_i = np.arange(8)
    r_i = np.arange(16)
    q_i = np.arange(32)
    pA = (t_i[:, None, None] * 512 + q_i[None, None, :] * 16
          + r_i[None, :, None]).reshape(128, 32)  # [v, q]
    hoA = (pA // WO).astype(np.float32)
    woA = (pA % WO).astype(np.float32)
    byA = hoA[:, None, None, :] + kh_of[None, None, :, None]  # [v, im, k, q]
    bxA = woA[:, None, None, :] + kw_of[None, None, :, None]
    byA = np.broadcast_to(byA, (128, 2, K, 32)).reshape(128, CCOL)
    bxA = np.broadcast_to(bxA, (128, 2, K, 32)).reshape(128, CCOL)
    byx = np.ascontiguousarray(np.concatenate([byA, bxA], axis=1))

    # layout A2 (weights): v = t*16 + kp, col = im*512 + (q*16+r)
    v_ar = np.arange(128)
    tB = v_ar // 16
    kpB = v_ar % 16
    kpc = np.minimum(kpB, K - 1)
    validB = (kpB < K).astype(np.float32)[:, None]  # [128, 1]
    c5 = np.arange(512)
    pB = tB[:, None] * 512 + c5[None, :]  # [128, 512]
    byB = ((pB // WO).astype(np.float32) + kh_of[kpc][:, None]) * validB
    bxB = ((pB % WO).astype(np.float32) + kw_of[kpc][:, None]) * validB
    byB2 = np.broadcast_to(byB[:, None, :], (128, 2, 512)).reshape(128, 1024)
    bxB2 = np.broadcast_to(bxB[:, None, :], (128, 2, 512)).reshape(128, 1024)
    byx2 = np.ascontiguousarray(np.concatenate([byB2, bxB2], axis=1))

    offr = off.reshape(N, K, 2, NPX)

    in_maps = []
    for core in range(NCORES):
        na, nb = 2 * core, 2 * core + 1
        imgs = [na, nb]

        # layout A dyx
        selA = offr[imgs][:, :, :, pA]  # [im, k, yx, v, q]
        dyA = selA[:, :, 0].transpose(2, 0, 1, 3).reshape(128, CCOL)
        dxA = selA[:, :, 1].transpose(2, 0, 1, 3).reshape(128, CCOL)
        dyx = np.ascontiguousarray(np.concatenate([dyA, dxA], axis=1))

        # layout A2 dyx2
        dyB = np.empty((128, 2, 512), np.float32)
        dxB = np.empty((128, 2, 512), np.float32)
        for imi, img in enumerate(imgs):
            dyB[:, imi] = offr[img][kpc[:, None], 0, pB] * validB
            dxB[:, imi] = offr[img][kpc[:, None], 1, pB] * validB
        dyx2 = np.ascontiguousarray(np.concatenate(
            [dyB.reshape(128, 1024), dxB.reshape(128, 1024)], axis=1))

        in_maps.append(dict(
            taba=tabs[na], tabb=tabs[nb], dyx=dyx, byx=byx,
            dyx2=dyx2, byx2=byx2, wmat2=wmat2, sel=sel,
        ))
    return in_maps


_NC_CACHE = None


def get_nc():
    global _NC_CACHE
    if _NC_CACHE is None:
        _NC_CACHE = build_bass()
    return _NC_CACHE


def kernel(input, offset, weight, _trace=False):
    from concourse.bass_utils import run_bass_kernel_spmd

    nc = get_nc()
    in_maps = _host_arrays(np.asarray(input), np.asarray(offset),
                           np.asarray(weight))
    res = run_bass_kernel_spmd(nc, in_maps, list(range(NCORES)), trace=_trace)
    out = np.empty((N, COUT, HO, WO), np.float32)
    for core in range(NCORES):
        o128 = np.asarray(res.results[core]["out128"])
        out[2 * core] = o128[0:64].reshape(COUT, HO, WO)
        out[2 * core + 1] = o128[64:128].reshape(COUT, HO, WO)
    if _trace:
        return out, res
    return out


# revision 31
# speedup vs baseline: 1.1611x; 1.1611x over previous
"""Deformable conv2d (torchvision semantics: stride=1, pad=0, dil=1,
offset_groups=1, no mask/bias) on 8 TRN2 NeuronCores via Bass/Tile.

Hardcoded shapes: input [16,64,66,66] f32, offset [16,18,64,64] f32,
weight [64,64,3,3] f32 -> out [16,64,64,64] f32.

Sharding: data-parallel over batch; core i handles images (2i, 2i+1).

Per-core scheme (per image):
  - HBM gather table per image: row m = tx*68+ty (tx-major), 128 bf16
    entries [c*2+x] = I[c, ty-1, tx-1+x] (zero-padded halo).  A SWDGE
    dma_gather (transpose mode) per (chunk t, image) fetches, for each
    of 4608 = 9 taps x 512 pixel indices, the 512B row-PAIR (rows m,
    m+1 = both y-corners), transposed so SBUF partitions carry (c,x)
    and the free dim carries (y-corner j, index position).
  - idx m = clamp(x0+1,0,67)*68 + clamp(y0+1,0,67); the zero halo makes
    every out-of-bounds corner read exactly 0, except the x1 slot when
    x0 < -1 and the second row when y0 < -1, which are killed in the
    bilinear weights (X1 *= [x0>=-1], Y1 *= [y0>=-1]).
  - bilinear corner weight products P[i,j] are computed compactly on
    DVE in a (t, k)-partition layout, staged contiguously through DRAM
    and read back as P_B [128=(ke,i,j,im,kh), 4096=(t,q,r)] so tiny
    one-hot contraction-64 matmuls broadcast them to the (c,x)
    partition layout (f32 PSUM), drained to bf16 by ScalarE.
  - DVE: S = G * W (bf16); TensorE: per tap 2 matmuls (j=0/1)
    contracting (c,x)=128, accumulating 9 taps in PSUM.
"""

import sys

sys.path.insert(0, "/opt/trn_rl_repo")

import ml_dtypes
import numpy as np

import concourse.bacc as bacc
import concourse.bass as bass
import concourse.mybir as mybir
import concourse.tile as tile

F32 = mybir.dt.float32
F32R = mybir.dt.float32r
BF16 = mybir.dt.bfloat16
I16 = mybir.dt.int16
I32 = mybir.dt.int32

N, CIN, COUT = 16, 64, 64
HIN, WIN = 66, 66
KH, KW = 3, 3
HO, WO = 64, 64
K = KH * KW
NPX = HO * WO  # 4096
NCORES = 8

TE = 68
NROW_PAD = 4640
NT = 8  # pixel chunks
CHUNK = 512
NIDX = K * CHUNK  # 4608 indices per (chunk, image) gather
CCOL = 2 * K * 32  # 576 idx-side compact cols: (im, k, qq)
N_COPY_KH = 3  # kh-blocks per (t,im) routed via Scalar PSUM->bf16 copy


def _alu(name):
    return getattr(mybir.AluOpType, name)


def build_bass(num_devices=NCORES):
    nc = bacc.Bacc("TRN2", target_bir_lowering=False, debug=False,
                   num_devices=num_devices)

    din = {}
    for nm, shp, dt in [
        ("taba", [NROW_PAD, 128], BF16), ("tabb", [NROW_PAD, 128], BF16),
        ("dyx", [128, 2 * CCOL], F32), ("byx", [128, 2 * CCOL], F32),
        ("dyx2", [128, 2048], F32), ("byx2", [128, 2048], F32),
        ("wmat2", [128, K * 64], BF16), ("sel", [128, 20 * 128], F32),
    ]:
        din[nm] = nc.dram_tensor(nm, shp, dt, kind="ExternalInput")
    out_d = nc.dram_tensor("out128", [128, NPX], F32, kind="ExternalOutput")
    istage = nc.dram_tensor("istage", [128 * CCOL], I16)
    wstage = nc.dram_tensor("wstage", [128 * 4096], BF16)

    # overlapping-window AP over each table: idx m -> 256 elems (rows m, m+1)
    tab_aps = [
        bass.AP(din[nm], 0, [[128, NROW_PAD - 1], [1, 256]])
        for nm in ("taba", "tabb")
    ]

    with tile.TileContext(nc) as tc:
        with tc.tile_pool(name="cst", bufs=1) as cpool:
            idxw = cpool.tile([128, 4608], I16, name="idxw")
            pb = cpool.tile([128, 4096], BF16, name="pb")
            wmat2 = cpool.tile([128, K * 64], BF16, name="wmat2_t")
            selt = cpool.tile([128, 20 * 128], BF16, name="sel_t")

            # ---------- phase 1: index + weight pipelines ----------
            with tc.tile_pool(name="pipe", bufs=1) as tp:
                sel_f = tp.tile([128, 20 * 128], F32, tag="sf", name="sel_f")
                nc.scalar.dma_start(sel_f[:], din["sel"].ap())
                nc.vector.tensor_copy(selt[:], sel_f[:])
                nc.sync.dma_start(wmat2[:], din["wmat2"].ap())
                nc.gpsimd.memset(pb[:], 0.0)

                def floor_(src, dst, nf, pool):
                    ti = pool.tile([128, nf], I32, tag="ti", name=f"ti{nf}")
                    nc.vector.tensor_copy(ti[:], src[:])
                    tf = pool.tile([128, nf], F32, tag="tf", name=f"tf{nf}")
                    nc.vector.tensor_copy(tf[:], ti[:])
                    co = pool.tile([128, nf], F32, tag="co", name=f"co{nf}")
                    nc.vector.tensor_tensor(co[:], tf[:], src[:],
                                            _alu("is_gt"))
                    nc.vector.tensor_sub(dst[:], tf[:], co[:])

                # --- idx chain, layout A: v=(t,r), col=(im,k,q) ---
                W2 = 2 * CCOL
                dyx = tp.tile([128, W2], F32, tag="dyx", name="dyx_t")
                nc.sync.dma_start(dyx[:], din["dyx"].ap())
                byx = tp.tile([128, W2], F32, tag="byx", name="byx_t")
                nc.sync.dma_start(byx[:], din["byx"].ap())
                pos = tp.tile([128, W2], F32, tag="pos", name="pos_t")
                nc.vector.tensor_add(pos[:], byx[:], dyx[:])
                fl = tp.tile([128, W2], F32, tag="fl", name="fl_t")
                floor_(pos, fl, W2, tp)
                tcl = tp.tile([128, W2], F32, tag="tcl", name="tcl_t")
                nc.vector.tensor_scalar(tcl[:], fl[:], 1.0, 67.0,
                                        _alu("add"), _alu("min"))
                nc.vector.tensor_scalar(tcl[:], tcl[:], 0.0, None, _alu("max"))
                mf = tp.tile([128, CCOL], F32, tag="mf", name="mf_t")
                nc.vector.tensor_scalar(mf[:], tcl[:, CCOL:W2], 68.0, None,
                                        _alu("mult"))
                nc.vector.tensor_add(mf[:], mf[:], tcl[:, 0:CCOL])
                icomp = tp.tile([128, CCOL], I16, tag="ic", name="icomp_t")
                nc.vector.tensor_copy(icomp[:], mf[:])

                nc.sync.dma_start(
                    istage.ap().rearrange("(v c) -> v c", c=CCOL), icomp[:])
                isrc = bass.AP(istage, 0, [[576, 16], [9216, 8], [1, 576]])
                for grp in range(8):
                    eng = nc.sync if grp % 2 == 0 else nc.scalar
                    eng.dma_start(idxw[grp * 16:(grp + 1) * 16, :], isrc)

                # --- weight chain, layout A2: v=(t,kp16), col=(im,q,r) ---
                W4 = 2048
                dyx2 = tp.tile([128, W4], F32, tag="dyx2", name="dyx2_t")
                nc.scalar.dma_start(dyx2[:], din["dyx2"].ap())
                byx2 = tp.tile([128, W4], F32, tag="byx2", name="byx2_t")
                nc.scalar.dma_start(byx2[:], din["byx2"].ap())
                pos2 = tp.tile([128, W4], F32, tag="pos2", name="pos2_t")
                nc.vector.tensor_add(pos2[:], byx2[:], dyx2[:])
                fl2 = tp.tile([128, W4], F32, tag="fl2", name="fl2_t")
                floor_(pos2, fl2, W4, tp)
                fr2 = tp.tile([128, W4], F32, tag="fr2", name="fr2_t")
                nc.vector.tensor_sub(fr2[:], pos2[:], fl2[:])
                msk = tp.tile([128, W4], F32, tag="msk", name="msk_t")
                nc.vector.tensor_scalar(msk[:], fl2[:], -1.0, None,
                                        _alu("is_ge"))
                f1 = tp.tile([128, W4], F32, tag="f1", name="f1_t")
                nc.vector.tensor_mul(f1[:], fr2[:], msk[:])
                f0 = tp.tile([128, W4], F32, tag="f0", name="f0_t")
                nc.vector.tensor_scalar(f0[:], fr2[:], -1.0, 1.0,
                                        _alu("mult"), _alu("add"))
                # products: prod[:, (i*2+j)*1024:+1024] = X_i * Y_j
                prod = tp.tile([128, 4096], BF16, tag="prod", name="prod_t")
                xs = [f0[:, 1024:2048], f1[:, 1024:2048]]
                ys = [f0[:, 0:1024], f1[:, 0:1024]]
                for i in range(2):
                    for j in range(2):
                        ij = i * 2 + j
                        nc.vector.tensor_tensor(
                            prod[:, ij * 1024:(ij + 1) * 1024],
                            xs[i], ys[j], _alu("mult"))

                nc.sync.dma_start(
                    wstage.ap().rearrange("(v c) -> v c", c=4096), prod[:])
                # reads: pb[ke*64+i*32+j*16+im*8+kh, (t, q, r)]
                # wstage flat: t:65536, kp:4096, i:2048, j:1024, im:512,
                #              q:16, r:1 ; k = 2*kh + ke -> kh stride 8192
                for ke in range(2):
                    nkh = 5 if ke == 0 else 4
                    for i in range(2):
                        for j in range(2):
                            for im in range(2):
                                base = ke * 64 + i * 32 + j * 16 + im * 8
                                off0 = (ke * 4096 + i * 2048 + j * 1024
                                        + im * 512)
                                src = bass.AP(
                                    wstage, off0,
                                    [[8192, nkh], [65536, 8], [1, 512]])
                                nc.sync.dma_start(pb[base:base + nkh, :], src)

            # ---------- phase 2: main loop ----------
            with tc.tile_pool(name="gat", bufs=20) as gpool, \
                 tc.tile_pool(name="smul", bufs=22) as spool, \
                 tc.tile_pool(name="wbp", bufs=4) as wbpool, \
                 tc.tile_pool(name="outs", bufs=2) as opool, \
                 tc.tile_pool(name="wps", bufs=3, space="PSUM") as wps, \
                 tc.tile_pool(name="ops", bufs=2, space="PSUM") as ops_:
                for t in range(NT):
                    for im in range(2):
                        gt = {}
                        for k in range(K):
                            g = gpool.tile([128, 2, CHUNK], BF16, tag="g",
                                           name=f"g_{t}_{im}_{k}")
                            nc.gpsimd.dma_gather(
                                g[:], tab_aps[im],
                                idxw[:, (t * 2 + im) * 288 + k * 32:
                                     (t * 2 + im) * 288 + (k + 1) * 32],
                                CHUNK, CHUNK, elem_size=256, elem_step=128,
                                transpose=True)
                            gt[k] = g
                        out_ps = ops_.tile([64, CHUNK], F32, tag="ops",
                                           name=f"ops_{t}_{im}")
                        S = {}
                        for kh in range(5):
                            nke = 2 if kh < 4 else 1
                            for j in range(2):
                                v20 = im * 10 + j * 5 + kh
                                wp = wps.tile([128, 2, CHUNK], F32, tag="wp",
                                              name=f"wp_{t}_{im}_{kh}_{j}")
                                for ke in range(nke):
                                    nc.tensor.matmul(
                                        wp[:, ke, :],
                                        selt[ke * 64:(ke + 1) * 64,
                                             v20 * 128:(v20 + 1) * 128],
                                        pb[ke * 64:(ke + 1) * 64,
                                           t * CHUNK:(t + 1) * CHUNK],
                                        start=True, stop=True)
                                if kh < N_COPY_KH:
                                    wpb = wbpool.tile(
                                        [128, 2, CHUNK], BF16, tag="wpb",
                                        name=f"wpb_{t}_{im}_{kh}_{j}")
                                    wpbf = wpb[:].rearrange(
                                        "p a b -> p (a b)")
                                    wpf = wp[:].rearrange("p a b -> p (a b)")
                                    nc.scalar.copy(
                                        wpbf[:, 0:nke * CHUNK],
                                        wpf[:, 0:nke * CHUNK])
                                    srcw = wpb
                                else:
                                    srcw = wp
                                for ke in range(nke):
                                    k = 2 * kh + ke
                                    s = spool.tile(
                                        [128, CHUNK], BF16, tag="s",
                                        name=f"s_{t}_{im}_{k}_{j}")
                                    nc.vector.tensor_tensor(
                                        s[:], gt[k][:, j, :],
                                        srcw[:, ke, :], _alu("mult"))
                                    S[(k, j)] = s
                        for kh in range(5):
                            nke = 2 if kh < 4 else 1
                            for ke in range(nke):
                                k = 2 * kh + ke
                                for j in range(2):
                                    nc.tensor.matmul(
                                        out_ps[:, :],
                                        wmat2[:, k * 64:(k + 1) * 64],
                                        S[(k, j)][:],
                                        start=(k == 0 and j == 0),
                                        stop=(k == 8 and j == 1))
                        ot = opool.tile([64, CHUNK], F32, tag="ot",
                                        name=f"ot_{t}_{im}")
                        nc.scalar.copy(ot[:], out_ps[:, :])
                        nc.sync.dma_start(
                            out_d.ap()[im * 64:(im + 1) * 64,
                                       t * CHUNK:(t + 1) * CHUNK], ot[:])

    nc.compile()
    return nc


# ---------------- host side ----------------

def _build_tables(inp):
    """inp [N, 64, 66, 66] f32 -> per-image [NROW_PAD, 128] bf16 tables."""
    n = inp.shape[0]
    ipad = np.zeros((n, 64, TE, TE + 1), np.float32)
    ipad[:, :, 1:1 + HIN, 1:1 + WIN] = inp
    # tab[n, tx, ty, c, i] = ipad[n, c, ty, tx+i]
    tab = np.empty((n, TE, TE, 64, 2), np.float32)
    tr = ipad.transpose(0, 3, 2, 1)  # [n, b(=tx+i), ty, c]
    tab[..., 0] = tr[:, 0:TE].transpose(0, 1, 2, 3)
    tab[..., 1] = tr[:, 1:TE + 1]
    full = np.zeros((n, NROW_PAD, 128), ml_dtypes.bfloat16)
    full[:, :TE * TE] = tab.reshape(n, TE * TE, 128).astype(
        ml_dtypes.bfloat16)
    return full


def _host_arrays(input, offset, weight):
    inp = np.ascontiguousarray(input, dtype=np.float32)
    off = np.ascontiguousarray(offset, dtype=np.float32)
    w = np.ascontiguousarray(weight, dtype=np.float32)

    tabs = _build_tables(inp)

    # wmat2[c*2+i, k*64+o] = w[o, c, kh, kw]
    wck_o = w.reshape(COUT, CIN, K).transpose(1, 2, 0)  # [c, k, o]
    wmat2 = np.broadcast_to(wck_o[:, None], (64, 2, K, 64))
    wmat2 = np.ascontiguousarray(wmat2.reshape(128, K * 64),
                                 dtype=ml_dtypes.bfloat16)

    # sel[z, v20*128 + u] = 1 iff z == (u%2)*32 + j*16 + im*8 + kh
    sel = np.zeros((64, 20, 128), np.float32)
    for im in range(2):
        for j in range(2):
            for kh in range(5):
                v20 = im * 10 + j * 5 + kh
                z0 = j * 16 + im * 8 + kh
                sel[z0, v20, 0::2] = 1.0
                sel[z0 + 32, v20, 1::2] = 1.0
    sel = sel.reshape(64, 20 * 128)
    sel = np.concatenate([sel, sel], axis=0)  # replicate for base 64 slices

    kh_of = (np.arange(K) // KW).astype(np.float32)
    kw_of = (np.arange(K) % KW).astype(np.float32)

    # layout A (idx): v = t*16 + r, col = im*288 + k*32 + q,
    # pixel p = t*512 + q*16 + r
    t_i = np.arange(8)
    r_i = np.arange(16)
    q_i = np.arange(32)
    pA = (t_i[:, None, None] * 512 + q_i[None, None, :] * 16
          + r_i[None, :, None]).reshape(128, 32)  # [v, q]
    hoA = (pA // WO).astype(np.float32)
    woA = (pA % WO).astype(np.float32)
    byA = hoA[:, None, None, :] + kh_of[None, None, :, None]  # [v, im, k, q]
    bxA = woA[:, None, None, :] + kw_of[None, None, :, None]
    byA = np.broadcast_to(byA, (128, 2, K, 32)).reshape(128, CCOL)
    bxA = np.broadcast_to(bxA, (128, 2, K, 32)).reshape(128, CCOL)
    byx = np.ascontiguousarray(np.concatenate([byA, bxA], axis=1))

    # layout A2 (weights): v = t*16 + kp, col = im*512 + (q*16+r)
    v_ar = np.arange(128)
    tB = v_ar // 16
    kpB = v_ar % 16
    kpc = np.minimum(kpB, K - 1)
    validB = (kpB < K).astype(np.float32)[:, None]  # [128, 1]
    c5 = np.arange(512)
    pB = tB[:, None] * 512 + c5[None, :]  # [128, 512]
    byB = ((pB // WO).astype(np.float32) + kh_of[kpc][:, None]) * validB
    bxB = ((pB % WO).astype(np.float32) + kw_of[kpc][:, None]) * validB
    byB2 = np.broadcast_to(byB[:, None, :], (128, 2, 512)).reshape(128, 1024)
    bxB2 = np.broadcast_to(bxB[:, None, :], (128, 2, 512)).reshape(128, 1024)
    byx2 = np.ascontiguousarray(np.concatenate([byB2, bxB2], axis=1))

    offr = off.reshape(N, K, 2, NPX)

    in_maps = []
    for core in range(NCORES):
        na, nb = 2 * core, 2 * core + 1
        imgs = [na, nb]

        # layout A dyx
        selA = offr[imgs][:, :, :, pA]  # [im, k, yx, v, q]
        dyA = selA[:, :, 0].transpose(2, 0, 1, 3).reshape(128, CCOL)
        dxA = selA[:, :, 1].transpose(2, 0, 1, 3).reshape(128, CCOL)
        dyx = np.ascontiguousarray(np.concatenate([dyA, dxA], axis=1))

        # layout A2 dyx2
        dyB = np.empty((128, 2, 512), np.float32)
        dxB = np.empty((128, 2, 512), np.float32)
        for imi, img in enumerate(imgs):
            dyB[:, imi] = offr[img][kpc[:, None], 0, pB] * validB
            dxB[:, imi] = offr[img][kpc[:, None], 1, pB] * validB
        dyx2 = np.ascontiguousarray(np.concatenate(
            [dyB.reshape(128, 1024), dxB.reshape(128, 1024)], axis=1))

        in_maps.append(dict(
            taba=tabs[na], tabb=tabs[nb], dyx=dyx, byx=byx,
            dyx2=dyx2, byx2=byx2, wmat2=wmat2, sel=sel,
        ))
    return in_maps


_NC_CACHE = None


def get_nc():
    global _NC_CACHE
    if _NC_CACHE is None:
        _NC_CACHE = build_bass()
    return _NC_CACHE


def kernel(input, offset, weight, _trace=False):
    from concourse.bass_utils import run_bass_kernel_spmd

    nc = get_nc()
    in_maps = _host_arrays(np.asarray(input), np.asarray(offset),
                           np.asarray(weight))
    res = run_bass_kernel_spmd(nc, in_maps, list(range(NCORES)), trace=_trace)
    out = np.empty((N, COUT, HO, WO), np.float32)
    for core in range(NCORES):
        o128 = np.asarray(res.results[core]["out128"])
        out[2 * core] = o128[0:64].reshape(COUT, HO, WO)
        out[2 * core + 1] = o128[64:128].reshape(COUT, HO, WO)
    if _trace:
        return out, res
    return out


# revision 32
# speedup vs baseline: 1.1722x; 1.0095x over previous
"""Deformable conv2d (torchvision semantics: stride=1, pad=0, dil=1,
offset_groups=1, no mask/bias) on 8 TRN2 NeuronCores via Bass/Tile.

Hardcoded shapes: input [16,64,66,66] f32, offset [16,18,64,64] f32,
weight [64,64,3,3] f32 -> out [16,64,64,64] f32.

Sharding: data-parallel over batch; core i handles images (2i, 2i+1).

Per-core scheme (per image):
  - HBM gather table per image: row m = tx*68+ty (tx-major), 128 bf16
    entries [c*2+x] = I[c, ty-1, tx-1+x] (zero-padded halo).  A SWDGE
    dma_gather (transpose mode) per (chunk t, image) fetches, for each
    of 4608 = 9 taps x 512 pixel indices, the 512B row-PAIR (rows m,
    m+1 = both y-corners), transposed so SBUF partitions carry (c,x)
    and the free dim carries (y-corner j, index position).
  - idx m = clamp(x0+1,0,67)*68 + clamp(y0+1,0,67); the zero halo makes
    every out-of-bounds corner read exactly 0, except the x1 slot when
    x0 < -1 and the second row when y0 < -1, which are killed in the
    bilinear weights (X1 *= [x0>=-1], Y1 *= [y0>=-1]).
  - bilinear corner weight products P[i,j] are computed compactly on
    DVE in a (t, k)-partition layout, staged contiguously through DRAM
    and read back as P_B [128=(ke,i,j,im,kh), 4096=(t,q,r)] so tiny
    one-hot contraction-64 matmuls broadcast them to the (c,x)
    partition layout (f32 PSUM), drained to bf16 by ScalarE.
  - DVE: S = G * W (bf16); TensorE: per tap 2 matmuls (j=0/1)
    contracting (c,x)=128, accumulating 9 taps in PSUM.
"""

import sys

sys.path.insert(0, "/opt/trn_rl_repo")

import ml_dtypes
import numpy as np

import concourse.bacc as bacc
import concourse.bass as bass
import concourse.mybir as mybir
import concourse.tile as tile

F32 = mybir.dt.float32
F32R = mybir.dt.float32r
BF16 = mybir.dt.bfloat16
I16 = mybir.dt.int16
I32 = mybir.dt.int32

N, CIN, COUT = 16, 64, 64
HIN, WIN = 66, 66
KH, KW = 3, 3
HO, WO = 64, 64
K = KH * KW
NPX = HO * WO  # 4096
NCORES = 8

TE = 68
NROW_PAD = 4640
NT = 8  # pixel chunks
CHUNK = 512
NIDX = K * CHUNK  # 4608 indices per (chunk, image) gather
CCOL = 2 * K * 32  # 576 idx-side compact cols: (im, k, qq)
N_COPY_KH = 3  # kh-blocks per (t,im) routed via Scalar PSUM->bf16 copy


def _alu(name):
    return getattr(mybir.AluOpType, name)


def build_bass(num_devices=NCORES):
    nc = bacc.Bacc("TRN2", target_bir_lowering=False, debug=False,
                   num_devices=num_devices)

    din = {}
    for nm, shp, dt in [
        ("taba", [NROW_PAD, 128], BF16), ("tabb", [NROW_PAD, 128], BF16),
        ("dyx", [128, 2 * CCOL], F32), ("byx", [128, 2 * CCOL], F32),
        ("dyx2", [128, 2048], F32), ("byx2", [128, 2048], F32),
        ("wmat2", [128, K * 64], BF16), ("sel", [128, 20 * 128], F32),
    ]:
        din[nm] = nc.dram_tensor(nm, shp, dt, kind="ExternalInput")
    out_d = nc.dram_tensor("out128", [128, NPX], F32, kind="ExternalOutput")
    istage = nc.dram_tensor("istage", [128 * CCOL], I16)
    wstage = nc.dram_tensor("wstage", [128 * 4096], BF16)

    # overlapping-window AP over each table: idx m -> 256 elems (rows m, m+1)
    tab_aps = [
        bass.AP(din[nm], 0, [[128, NROW_PAD - 1], [1, 256]])
        for nm in ("taba", "tabb")
    ]

    with tile.TileContext(nc) as tc:
        with tc.tile_pool(name="cst", bufs=1) as cpool:
            idxw = cpool.tile([128, 4608], I16, name="idxw")
            pb = cpool.tile([128, 4096], BF16, name="pb")
            wmat2 = cpool.tile([128, K * 64], BF16, name="wmat2_t")
            selt = cpool.tile([128, 20 * 128], BF16, name="sel_t")

            # ---------- phase 1: index + weight pipelines ----------
            with tc.tile_pool(name="pipe", bufs=1) as tp:
                sel_f = tp.tile([128, 20 * 128], F32, tag="sf", name="sel_f")
                nc.sync.dma_start(sel_f[:], din["sel"].ap())
                nc.vector.tensor_copy(selt[:], sel_f[:])
                nc.sync.dma_start(wmat2[:], din["wmat2"].ap())
                nc.gpsimd.memset(pb[:], 0.0)

                def floor_(src, dst, nf, pool):
                    ti = pool.tile([128, nf], I32, tag="ti", name=f"ti{nf}")
                    nc.vector.tensor_copy(ti[:], src[:])
                    tf = pool.tile([128, nf], F32, tag="tf", name=f"tf{nf}")
                    nc.vector.tensor_copy(tf[:], ti[:])
                    co = pool.tile([128, nf], F32, tag="co", name=f"co{nf}")
                    nc.vector.tensor_tensor(co[:], tf[:], src[:],
                                            _alu("is_gt"))
                    nc.vector.tensor_sub(dst[:], tf[:], co[:])

                # --- idx chain, layout A: v=(t,r), col=(im,k,q) ---
                W2 = 2 * CCOL
                dyx = tp.tile([128, W2], F32, tag="dyx", name="dyx_t")
                nc.sync.dma_start(dyx[:], din["dyx"].ap())
                byx = tp.tile([128, W2], F32, tag="byx", name="byx_t")
                nc.sync.dma_start(byx[:], din["byx"].ap())
                pos = tp.tile([128, W2], F32, tag="pos", name="pos_t")
                nc.vector.tensor_add(pos[:], byx[:], dyx[:])
                fl = tp.tile([128, W2], F32, tag="fl", name="fl_t")
                floor_(pos, fl, W2, tp)
                tcl = tp.tile([128, W2], F32, tag="tcl", name="tcl_t")
                nc.vector.tensor_scalar(tcl[:], fl[:], 1.0, 67.0,
                                        _alu("add"), _alu("min"))
                nc.vector.tensor_scalar(tcl[:], tcl[:], 0.0, None, _alu("max"))
                mf = tp.tile([128, CCOL], F32, tag="mf", name="mf_t")
                nc.vector.tensor_scalar(mf[:], tcl[:, CCOL:W2], 68.0, None,
                                        _alu("mult"))
                nc.vector.tensor_add(mf[:], mf[:], tcl[:, 0:CCOL])
                icomp = tp.tile([128, CCOL], I16, tag="ic", name="icomp_t")
                nc.vector.tensor_copy(icomp[:], mf[:])

                nc.sync.dma_start(
                    istage.ap().rearrange("(v c) -> v c", c=CCOL), icomp[:])
                isrc = bass.AP(istage, 0, [[576, 16], [9216, 8], [1, 576]])
                for grp in range(8):
                    nc.sync.dma_start(idxw[grp * 16:(grp + 1) * 16, :], isrc)

                # --- weight chain, layout A2: v=(t,kp16), col=(im,q,r) ---
                W4 = 2048
                dyx2 = tp.tile([128, W4], F32, tag="dyx2", name="dyx2_t")
                nc.sync.dma_start(dyx2[:], din["dyx2"].ap())
                byx2 = tp.tile([128, W4], F32, tag="byx2", name="byx2_t")
                nc.sync.dma_start(byx2[:], din["byx2"].ap())
                pos2 = tp.tile([128, W4], F32, tag="pos2", name="pos2_t")
                nc.vector.tensor_add(pos2[:], byx2[:], dyx2[:])
                fl2 = tp.tile([128, W4], F32, tag="fl2", name="fl2_t")
                floor_(pos2, fl2, W4, tp)
                fr2 = tp.tile([128, W4], F32, tag="fr2", name="fr2_t")
                nc.vector.tensor_sub(fr2[:], pos2[:], fl2[:])
                msk = tp.tile([128, W4], F32, tag="msk", name="msk_t")
                nc.vector.tensor_scalar(msk[:], fl2[:], -1.0, None,
                                        _alu("is_ge"))
                f1 = tp.tile([128, W4], F32, tag="f1", name="f1_t")
                nc.vector.tensor_mul(f1[:], fr2[:], msk[:])
                f0 = tp.tile([128, W4], F32, tag="f0", name="f0_t")
                nc.vector.tensor_scalar(f0[:], fr2[:], -1.0, 1.0,
                                        _alu("mult"), _alu("add"))
                # products: prod[:, (i*2+j)*1024:+1024] = X_i * Y_j
                prod = tp.tile([128, 4096], BF16, tag="prod", name="prod_t")
                xs = [f0[:, 1024:2048], f1[:, 1024:2048]]
                ys = [f0[:, 0:1024], f1[:, 0:1024]]
                for i in range(2):
                    for j in range(2):
                        ij = i * 2 + j
                        nc.vector.tensor_tensor(
                            prod[:, ij * 1024:(ij + 1) * 1024],
                            xs[i], ys[j], _alu("mult"))

                nc.sync.dma_start(
                    wstage.ap().rearrange("(v c) -> v c", c=4096), prod[:])
                # reads: pb[ke*64+i*32+j*16+im*8+kh, (t, q, r)]
                # wstage flat: t:65536, kp:4096, i:2048, j:1024, im:512,
                #              q:16, r:1 ; k = 2*kh + ke -> kh stride 8192
                for ke in range(2):
                    nkh = 5 if ke == 0 else 4
                    for i in range(2):
                        for j in range(2):
                            for im in range(2):
                                base = ke * 64 + i * 32 + j * 16 + im * 8
                                off0 = (ke * 4096 + i * 2048 + j * 1024
                                        + im * 512)
                                src = bass.AP(
                                    wstage, off0,
                                    [[8192, nkh], [65536, 8], [1, 512]])
                                nc.sync.dma_start(pb[base:base + nkh, :], src)

            # ---------- phase 2: main loop ----------
            with tc.tile_pool(name="gat", bufs=20) as gpool, \
                 tc.tile_pool(name="smul", bufs=22) as spool, \
                 tc.tile_pool(name="wbp", bufs=4) as wbpool, \
                 tc.tile_pool(name="outs", bufs=2) as opool, \
                 tc.tile_pool(name="wps", bufs=3, space="PSUM") as wps, \
                 tc.tile_pool(name="ops", bufs=2, space="PSUM") as ops_:
                for t in range(NT):
                    for im in range(2):
                        gt = {}
                        for k in range(K):
                            g = gpool.tile([128, 2, CHUNK], BF16, tag="g",
                                           name=f"g_{t}_{im}_{k}")
                            nc.gpsimd.dma_gather(
                                g[:], tab_aps[im],
                                idxw[:, (t * 2 + im) * 288 + k * 32:
                                     (t * 2 + im) * 288 + (k + 1) * 32],
                                CHUNK, CHUNK, elem_size=256, elem_step=128,
                                transpose=True)
                            gt[k] = g
                        out_ps = ops_.tile([64, CHUNK], F32, tag="ops",
                                           name=f"ops_{t}_{im}")
                        S = {}
                        for kh in range(5):
                            nke = 2 if kh < 4 else 1
                            for j in range(2):
                                v20 = im * 10 + j * 5 + kh
                                wp = wps.tile([128, 2, CHUNK], F32, tag="wp",
                                              name=f"wp_{t}_{im}_{kh}_{j}")
                                for ke in range(nke):
                                    nc.tensor.matmul(
                                        wp[:, ke, :],
                                        selt[ke * 64:(ke + 1) * 64,
                                             v20 * 128:(v20 + 1) * 128],
                                        pb[ke * 64:(ke + 1) * 64,
                                           t * CHUNK:(t + 1) * CHUNK],
                                        start=True, stop=True)
                                if kh < N_COPY_KH:
                                    wpb = wbpool.tile(
                                        [128, 2, CHUNK], BF16, tag="wpb",
                                        name=f"wpb_{t}_{im}_{kh}_{j}")
                                    wpbf = wpb[:].rearrange(
                                        "p a b -> p (a b)")
                                    wpf = wp[:].rearrange("p a b -> p (a b)")
                                    nc.scalar.copy(
                                        wpbf[:, 0:nke * CHUNK],
                                        wpf[:, 0:nke * CHUNK])
                                    srcw = wpb
                                else:
                                    srcw = wp
                                for ke in range(nke):
                                    k = 2 * kh + ke
                                    s = spool.tile(
                                        [128, CHUNK], BF16, tag="s",
                                        name=f"s_{t}_{im}_{k}_{j}")
                                    nc.vector.tensor_tensor(
                                        s[:], gt[k][:, j, :],
                                        srcw[:, ke, :], _alu("mult"))
                                    S[(k, j)] = s
                        for kh in range(5):
                            nke = 2 if kh < 4 else 1
                            for ke in range(nke):
                                k = 2 * kh + ke
                                for j in range(2):
                                    nc.tensor.matmul(
                                        out_ps[:, :],
                                        wmat2[:, k * 64:(k + 1) * 64],
                                        S[(k, j)][:],
                                        start=(k == 0 and j == 0),
                                        stop=(k == 8 and j == 1))
                        ot = opool.tile([64, CHUNK], F32, tag="ot",
                                        name=f"ot_{t}_{im}")
                        nc.scalar.copy(ot[:], out_ps[:, :])
                        nc.sync.dma_start(
                            out_d.ap()[im * 64:(im + 1) * 64,
                                       t * CHUNK:(t + 1) * CHUNK], ot[:])

    nc.compile()
    return nc


# ---------------- host side ----------------

def _build_tables(inp):
    """inp [N, 64, 66, 66] f32 -> per-image [NROW_PAD, 128] bf16 tables."""
    n = inp.shape[0]
    ipad = np.zeros((n, 64, TE, TE + 1), np.float32)
    ipad[:, :, 1:1 + HIN, 1:1 + WIN] = inp
    # tab[n, tx, ty, c, i] = ipad[n, c, ty, tx+i]
    tab = np.empty((n, TE, TE, 64, 2), np.float32)
    tr = ipad.transpose(0, 3, 2, 1)  # [n, b(=tx+i), ty, c]
    tab[..., 0] = tr[:, 0:TE].transpose(0, 1, 2, 3)
    tab[..., 1] = tr[:, 1:TE + 1]
    full = np.zeros((n, NROW_PAD, 128), ml_dtypes.bfloat16)
    full[:, :TE * TE] = tab.reshape(n, TE * TE, 128).astype(
        ml_dtypes.bfloat16)
    return full


def _host_arrays(input, offset, weight):
    inp = np.ascontiguousarray(input, dtype=np.float32)
    off = np.ascontiguousarray(offset, dtype=np.float32)
    w = np.ascontiguousarray(weight, dtype=np.float32)

    tabs = _build_tables(inp)

    # wmat2[c*2+i, k*64+o] = w[o, c, kh, kw]
    wck_o = w.reshape(COUT, CIN, K).transpose(1, 2, 0)  # [c, k, o]
    wmat2 = np.broadcast_to(wck_o[:, None], (64, 2, K, 64))
    wmat2 = np.ascontiguousarray(wmat2.reshape(128, K * 64),
                                 dtype=ml_dtypes.bfloat16)

    # sel[z, v20*128 + u] = 1 iff z == (u%2)*32 + j*16 + im*8 + kh
    sel = np.zeros((64, 20, 128), np.float32)
    for im in range(2):
        for j in range(2):
            for kh in range(5):
                v20 = im * 10 + j * 5 + kh
                z0 = j * 16 + im * 8 + kh
                sel[z0, v20, 0::2] = 1.0
                sel[z0 + 32, v20, 1::2] = 1.0
    sel = sel.reshape(64, 20 * 128)
    sel = np.concatenate([sel, sel], axis=0)  # replicate for base 64 slices

    kh_of = (np.arange(K) // KW).astype(np.float32)
    kw_of = (np.arange(K) % KW).astype(np.float32)

    # layout A (idx): v = t*16 + r, col = im*288 + k*32 + q,
    # pixel p = t*512 + q*16 + r
    t_i = np.arange(8)
    r_i = np.arange(16)
    q_i = np.arange(32)
    pA = (t_i[:, None, None] * 512 + q_i[None, None, :] * 16
          + r_i[None, :, None]).reshape(128, 32)  # [v, q]
    hoA = (pA // WO).astype(np.float32)
    woA = (pA % WO).astype(np.float32)
    byA = hoA[:, None, None, :] + kh_of[None, None, :, None]  # [v, im, k, q]
    bxA = woA[:, None, None, :] + kw_of[None, None, :, None]
    byA = np.broadcast_to(byA, (128, 2, K, 32)).reshape(128, CCOL)
    bxA = np.broadcast_to(bxA, (128, 2, K, 32)).reshape(128, CCOL)
    byx = np.ascontiguousarray(np.concatenate([byA, bxA], axis=1))

    # layout A2 (weights): v = t*16 + kp, col = im*512 + (q*16+r)
    v_ar = np.arange(128)
    tB = v_ar // 16
    kpB = v_ar % 16
    kpc = np.minimum(kpB, K - 1)
    validB = (kpB < K).astype(np.float32)[:, None]  # [128, 1]
    c5 = np.arange(512)
    pB = tB[:, None] * 512 + c5[None, :]  # [128, 512]
    byB = ((pB // WO).astype(np.float32) + kh_of[kpc][:, None]) * validB
    bxB = ((pB % WO).astype(np.float32) + kw_of[kpc][:, None]) * validB
    byB2 = np.broadcast_to(byB[:, None, :], (128, 2, 512)).reshape(128, 1024)
    bxB2 = np.broadcast_to(bxB[:, None, :], (128, 2, 512)).reshape(128, 1024)
    byx2 = np.ascontiguousarray(np.concatenate([byB2, bxB2], axis=1))

    offr = off.reshape(N, K, 2, NPX)

    in_maps = []
    for core in range(NCORES):
        na, nb = 2 * core, 2 * core + 1
        imgs = [na, nb]

        # layout A dyx
        selA = offr[imgs][:, :, :, pA]  # [im, k, yx, v, q]
        dyA = selA[:, :, 0].transpose(2, 0, 1, 3).reshape(128, CCOL)
        dxA = selA[:, :, 1].transpose(2, 0, 1, 3).reshape(128, CCOL)
        dyx = np.ascontiguousarray(np.concatenate([dyA, dxA], axis=1))

        # layout A2 dyx2
        dyB = np.empty((128, 2, 512), np.float32)
        dxB = np.empty((128, 2, 512), np.float32)
        for imi, img in enumerate(imgs):
            dyB[:, imi] = offr[img][kpc[:, None], 0, pB] * validB
            dxB[:, imi] = offr[img][kpc[:, None], 1, pB] * validB
        dyx2 = np.ascontiguousarray(np.concatenate(
            [dyB.reshape(128, 1024), dxB.reshape(128, 1024)], axis=1))

        in_maps.append(dict(
            taba=tabs[na], tabb=tabs[nb], dyx=dyx, byx=byx,
            dyx2=dyx2, byx2=byx2, wmat2=wmat2, sel=sel,
        ))
    return in_maps


_NC_CACHE = None


def get_nc():
    global _NC_CACHE
    if _NC_CACHE is None:
        _NC_CACHE = build_bass()
    return _NC_CACHE


def kernel(input, offset, weight, _trace=False):
    from concourse.bass_utils import run_bass_kernel_spmd

    nc = get_nc()
    in_maps = _host_arrays(np.asarray(input), np.asarray(offset),
                           np.asarray(weight))
    res = run_bass_kernel_spmd(nc, in_maps, list(range(NCORES)), trace=_trace)
    out = np.empty((N, COUT, HO, WO), np.float32)
    for core in range(NCORES):
        o128 = np.asarray(res.results[core]["out128"])
        out[2 * core] = o128[0:64].reshape(COUT, HO, WO)
        out[2 * core + 1] = o128[64:128].reshape(COUT, HO, WO)
    if _trace:
        return out, res
    return out
